# revision 14
# baseline (speedup 1.0000x reference)
"""Trainium2 Bass kernel for nn_NormalizedCausalAttention.

Full (unsharded) inputs in, full outputs out. Internally shards across 8
NeuronCores: data-parallel over batch (B=2), tensor-parallel over heads
(16 heads -> 4 per core).

Per-core device program (core c -> batch b=c//4, heads hb=4*(c%4)..+4):
  Phase 1: Q/K/V projections.  qT/kT stored [64, h, S] (head dim on
           partitions 0-63), v stored [S, d] augmented with a ones column
           per head (so attn@V also produces softmax row sums).
  Phase 2: per q-tile of 256 and k-tile of 128 (causal tiles only):
           scoresT[k,q] = kT.T@qT -> +causal bias (DVE) -> exp (ACT) ->
           attn@V accumulate in PSUM (PE); row sums ride as output row 64.
           Unnormalized exp tiles are written to DRAM in [k,q] orientation;
           reciprocal row sums are exported.
  Phase 3: out-projection partials from the normalized attn output.
Host: sums the 4 per-batch partial outputs (+bo), and transposes/normalizes
the attention-weight planes (attn = (expT * recip[q]).T).

Masking: host folds attn_mask into the causal bias (NEG where masked), so
exp underflows to exactly 0.0 like the reference softmax.

Hardware notes:
 - all K=64 matmuls keep both operands on partitions 0-63: alternating PE
   row groups (tile_position (0,0)/(64,0)) crashes the device on this stack.
 - walrus here accepts only ONE semaphore wait per instruction; the
   _split_multiwaits post-pass hoists extra waits onto NoOps.
"""

import sys

for _p in ("/opt/trn_rl_repo",):
    if _p not in sys.path:
        sys.path.insert(0, _p)

import numpy as np

import concourse.bass as bass
import concourse.mybir as mybir
import concourse.tile as tile
from concourse.bass_utils import run_bass_kernel_spmd

B, S, E, H = 2, 2048, 1024, 16
HD = E // H  # 64
HPC = 4      # heads per core
NCORES = 8
NEG = -1.0e9
TEMPERATURE = 1.0
FP = mybir.dt.float32
QT = 256               # q tile width in phase 2
NQT = S // QT          # 8
AF = mybir.ActivationFunctionType


def _split_multiwaits(nc):
    """Walrus here rejects >1 sem wait per instruction: hoist extras onto
    NoOps inserted right before the instruction on the same engine."""
    for f in nc.m.functions:
        for blk in f.blocks:
            out = []
            changed = False
            for inst in blk.instructions:
                si = inst.sync_info
                if si is not None and si.on_wait is not None and len(si.on_wait) > 1:
                    waits = list(si.on_wait)
                    for i, w in enumerate(waits[:-1]):
                        out.append(mybir.InstNoOp(
                            name=f"{inst.name}-wsplit{i}",
                            engine=inst.engine,
                            ins=[], outs=[],
                            sync_info=mybir.SyncInfo(on_wait=[w], on_update=[]),
                        ))
                    si.on_wait = waits[-1:]
                    inst.sync_info = si
                    changed = True
                out.append(inst)
            if changed:
                blk.instructions = out


def _nkt(qtj):
    # number of 128-wide k tiles needed for q in [QT*qtj, QT*qtj+QT)
    return (QT * qtj + QT - 1) // 128 + 1


def _build_nc(split_waits=True, phases=(1, 2, 3)):
    nc = bass.Bass()

    xq = nc.dram_tensor("xq", [E, S], FP, kind="ExternalInput")   # query[b].T
    xk = nc.dram_tensor("xk", [E, S], FP, kind="ExternalInput")   # key_t[b].T
    xv = nc.dram_tensor("xv", [E, S], FP, kind="ExternalInput")   # value[b].T
    wq = nc.dram_tensor("wq", [E, 256], FP, kind="ExternalInput")  # (Wq[h]/8).T
    wk = nc.dram_tensor("wk", [E, 256], FP, kind="ExternalInput")
    wv = nc.dram_tensor("wv", [E, 256], FP, kind="ExternalInput")
    wo = nc.dram_tensor("wo", [64, HPC, E], FP, kind="ExternalInput")
    bq = nc.dram_tensor("bq", [64, HPC], FP, kind="ExternalInput")
    bk = nc.dram_tensor("bk", [64, HPC], FP, kind="ExternalInput")
    bv = nc.dram_tensor("bv", [1, 256], FP, kind="ExternalInput")
    cwt = nc.dram_tensor("cwt", [S, S], FP, kind="ExternalInput")  # masked cw[b].T

    awt = nc.dram_tensor("awt", [HPC, S, S], FP, kind="ExternalOutput")
    outp = nc.dram_tensor("outp", [S, E], FP, kind="ExternalOutput")
    rsum = nc.dram_tensor("rsum", [1, HPC * S], FP, kind="ExternalOutput")

    with tile.TileContext(nc) as tc:
        with tc.tile_pool(name="const", bufs=1) as cpool, \
             tc.tile_pool(name="qkv", bufs=1) as qkvpool:

            ones_row = cpool.tile([1, 128], FP)
            nc.vector.memset(ones_row, 1.0)
            bvr_sb = cpool.tile([1, 256], FP)
            nc.sync.dma_start(out=bvr_sb, in_=bv[:, :])

            qT_sb = qkvpool.tile([64, HPC, S], FP)
            kT_sb = qkvpool.tile([64, HPC, S], FP)
            v_sb = qkvpool.tile([128, S // 128, HPC * 65], FP)
            outT_sb = qkvpool.tile([64, HPC, S], FP)

            # ones columns of the augmented v (row-sum trick)
            v4 = v_sb.rearrange("p s (h x) -> p s h x", x=65)
            nc.vector.memset(v4[:, :, :, 64:65], 1.0)

            # ---------------- Phase 1: projections ----------------
            if 1 not in phases:
                nc.vector.memset(qT_sb, 0.01)
                nc.vector.memset(kT_sb, 0.01)
                nc.vector.memset(v4[:, :, :, 0:64], 0.01)
                nc.vector.memset(outT_sb, 0.01)
            if 1 in phases:
              with tc.tile_pool(name="xp", bufs=1) as xpool, \
                   tc.tile_pool(name="wp", bufs=1) as wpool, \
                   tc.tile_pool(name="pp", bufs=2, space="PSUM") as ppsum:

                bq_sb = wpool.tile([64, HPC], FP, tag="bias")
                nc.sync.dma_start(out=bq_sb, in_=bq[:, :])
                bk_sb = wpool.tile([64, HPC], FP, tag="bias")
                nc.sync.dma_start(out=bk_sb, in_=bk[:, :])

                for (xdram, wdram, b_sb, dst) in (
                    (xq, wq, bq_sb, qT_sb),
                    (xk, wk, bk_sb, kT_sb),
                ):
                    w_sb = wpool.tile([128, 8, 256], FP, tag="w")
                    nc.sync.dma_start(
                        out=w_sb, in_=wdram[:, :].rearrange("(c p) n -> p c n", p=128)
                    )
                    x_sb = xpool.tile([128, 8, S], FP, tag="x")
                    nc.sync.dma_start(
                        out=x_sb, in_=xdram[:, :].rearrange("(c p) s -> p c s", p=128)
                    )
                    for h in range(HPC):
                        for st in range(4):
                            ps = ppsum.tile([64, 512], FP, tag="pqk")
                            for c in range(8):
                                nc.tensor.matmul(
                                    ps,
                                    w_sb[:, c, 64 * h:64 * h + 64],
                                    x_sb[:, c, 512 * st:512 * st + 512],
                                    start=(c == 0),
                                    stop=(c == 7),
                                )
                            nc.scalar.activation(
                                dst[:, h, 512 * st:512 * st + 512],
                                ps,
                                AF.Identity,
                                bias=b_sb[:, h:h + 1],
                                scale=1.0,
                            )

                # v projection: v[s, d] with per-head ones columns
                wv_sb = wpool.tile([128, 8, 256], FP, tag="w")
                nc.sync.dma_start(
                    out=wv_sb, in_=wv[:, :].rearrange("(c p) n -> p c n", p=128)
                )
                xv_sb = xpool.tile([128, 8, S], FP, tag="x")
                nc.sync.dma_start(
                    out=xv_sb, in_=xv[:, :].rearrange("(c p) s -> p c s", p=128)
                )
                for st in range(S // 128):
                    ps = ppsum.tile([128, 256], FP, tag="pv")
                    for c in range(8):
                        nc.tensor.matmul(
                            ps,
                            xv_sb[:, c, 128 * st:128 * st + 128],
                            wv_sb[:, c, :],
                            start=(c == 0),
                            stop=False,
                        )
                    # + bv (outer product with ones)
                    nc.tensor.matmul(ps, ones_row, bvr_sb, start=False, stop=True)
                    nc.scalar.copy(
                        v4[:, st, :, 0:64],
                        ps[:, :].rearrange("p (h x) -> p h x", x=64),
                    )

            # ---------------- Phase 2: attention ----------------
            if 2 in phases:
              with tc.tile_pool(name="cw", bufs=1) as cwpool, \
                   tc.tile_pool(name="ep", bufs=2) as epool, \
                   tc.tile_pool(name="rcp", bufs=6) as rcppool, \
                   tc.tile_pool(name="rb", bufs=2) as rbpool, \
                   tc.tile_pool(name="sps", bufs=2, space="PSUM") as spspool, \
                   tc.tile_pool(name="ops", bufs=4, space="PSUM") as opspool, \
                   tc.tile_pool(name="rbp", bufs=2, space="PSUM") as rbppool:

                for qtj in range(NQT):
                    nkt = _nkt(qtj)
                    q0 = QT * qtj
                    e_ts = [epool.tile([128, nkt, 2, QT], FP, tag="e",
                                       name=f"e{qtj}_{p2}")
                            for p2 in range(2)]
                    o_ps = [opspool.tile([65, QT], FP, tag="o", name=f"o{qtj}_{h}")
                            for h in range(HPC)]
                    cw_t2 = cwpool.tile([128, 16, QT], FP, tag="cw", name=f"cw{qtj}")
                    nc.sync.dma_start(
                        out=cw_t2[:, 0:nkt, :],
                        in_=cwt[0:128 * nkt, q0:q0 + QT].rearrange(
                            "(k p) q -> p k q", p=128
                        ),
                    )
                    for kt in range(nkt):
                        cw_ap = cw_t2[:, kt, :]
                        cw_b2 = bass.AP(
                            tensor=cw_ap.tensor,
                            offset=cw_ap.offset,
                            ap=[list(cw_ap.ap[0]), [0, 2], list(cw_ap.ap[1])],
                        )
                        for p2 in range(2):
                            sp = spspool.tile([128, 2, QT], FP, tag="s",
                                              name=f"sp{qtj}_{kt}_{p2}")
                            for j2 in range(2):
                                h = 2 * p2 + j2
                                nc.tensor.matmul(
                                    sp[:, j2, :],
                                    kT_sb[:, h, 128 * kt:128 * kt + 128],
                                    qT_sb[:, h, q0:q0 + QT],
                                    start=True,
                                    stop=True,
                                )
                            # + causal bias, both heads at once
                            nc.vector.tensor_add(e_ts[p2][:, kt, :, :], sp, cw_b2)
                            nc.scalar.activation(
                                e_ts[p2][:, kt, :, :], e_ts[p2][:, kt, :, :], AF.Exp
                            )
                            for j2 in range(2):
                                h = 2 * p2 + j2
                                nc.tensor.matmul(
                                    o_ps[h],
                                    v_sb[:, kt, 65 * h:65 * h + 65],
                                    e_ts[p2][:, kt, j2, :],
                                    start=(kt == 0),
                                    stop=(kt == nkt - 1),
                                )
                    for p2 in range(2):
                        for j2 in range(2):
                            h = 2 * p2 + j2
                            rc = rcppool.tile([1, QT], FP, tag="rc",
                                              name=f"rc{qtj}_{h}")
                            nc.vector.reciprocal(rc, o_ps[h][64:65, :])
                            nc.sync.dma_start(
                                out=rsum[:, h * S + q0:h * S + q0 + QT], in_=rc
                            )
                            rb_ps = rbppool.tile([128, QT], FP, tag="rb",
                                                 name=f"rb{qtj}_{h}")
                            nc.tensor.matmul(rb_ps, ones_row, rc, start=True,
                                             stop=True)
                            rb_sb = rbpool.tile([128, QT], FP, tag="rbs",
                                                name=f"rbs{qtj}_{h}")
                            nc.scalar.copy(rb_sb, rb_ps)
                            # normalized attention output (pre out-projection)
                            nc.vector.tensor_mul(
                                outT_sb[:, h, q0:q0 + QT],
                                o_ps[h][0:64, :],
                                rb_sb[0:64, :],
                            )
                            # unnormalized exp tile block -> DRAM ([k, q] layout)
                            nc.sync.dma_start(
                                out=awt[h, 0:128 * nkt, q0:q0 + QT].rearrange(
                                    "(k p) q -> p k q", p=128
                                ),
                                in_=e_ts[p2][:, :, j2, :],
                            )

            # ---------------- Phase 3: out projection ----------------
            if 3 in phases:
              with tc.tile_pool(name="op", bufs=3) as opool, \
                   tc.tile_pool(name="wop", bufs=1) as wopool, \
                   tc.tile_pool(name="po", bufs=2, space="PSUM") as popsum:
                wo_sb = wopool.tile([64, HPC, E], FP)
                nc.sync.dma_start(out=wo_sb, in_=wo[:, :, :])
                for st in range(S // 128):
                    for et in range(2):
                        ps = popsum.tile([128, 512], FP, tag="po")
                        for h in range(HPC):
                            nc.tensor.matmul(
                                ps,
                                outT_sb[:, h, 128 * st:128 * st + 128],
                                wo_sb[:, h, 512 * et:512 * et + 512],
                                start=(h == 0),
                                stop=(h == HPC - 1),
                            )
                        ot = opool.tile([128, 512], FP, tag="ot")
                        nc.scalar.copy(ot, ps)
                        nc.sync.dma_start(
                            out=outp[128 * st:128 * st + 128,
                                     512 * et:512 * et + 512],
                            in_=ot,
                        )
    if split_waits:
        _split_multiwaits(nc)
    return nc


_NC_CACHE = None
PROFILE = False           # set by test harness; harness-safe default off
LAST_RESULTS = None       # BassKernelResults of the last run (for profiling)


def _get_nc():
    global _NC_CACHE
    if _NC_CACHE is None:
        _NC_CACHE = _build_nc()
    return _NC_CACHE


def kernel(query, key_t, value, causal_weights, attn_mask,
           Wq, bq, Wk, bk, Wv, bv, Wo, bo, Wc, bc, **_unused):
    query = np.asarray(query, np.float32)
    key_t = np.asarray(key_t, np.float32)
    value = np.asarray(value, np.float32)
    causal_weights = np.asarray(causal_weights, np.float32)
    attn_mask = np.asarray(attn_mask)
    Wq = np.asarray(Wq, np.float32)
    Wk = np.asarray(Wk, np.float32)
    Wv = np.asarray(Wv, np.float32)
    Wo = np.asarray(Wo, np.float32)
    bq = np.asarray(bq, np.float32)
    bk = np.asarray(bk, np.float32)
    bv = np.asarray(bv, np.float32)
    bo = np.asarray(bo, np.float32)

    sc = 1.0 / np.sqrt(np.float32(HD))

    # per-batch shared tensors
    xqT, xkT, xvT, cwmT = {}, {}, {}, {}
    for b in range(B):
        xqT[b] = np.ascontiguousarray(query[b].T)
        xkT[b] = np.ascontiguousarray(key_t[b].T)
        xvT[b] = np.ascontiguousarray(value[b].T)
        cwm = np.where(attn_mask[b] != 0,
                       causal_weights[b] / np.float32(TEMPERATURE),
                       np.float32(NEG)).astype(np.float32)
        cwmT[b] = np.ascontiguousarray(cwm.T)

    in_maps = []
    for c in range(NCORES):
        b = c // 4
        hb = HPC * (c % 4)
        rows = slice(hb * HD, (hb + HPC) * HD)  # 256 projection dims
        wq_c = np.ascontiguousarray((Wq[rows] * sc).T)
        wk_c = np.ascontiguousarray(Wk[rows].T)
        wv_c = np.ascontiguousarray(Wv[rows].T)
        # wo[d, h, e] = Wo[e, hb*64 + h*64 + d]
        wo_c = np.ascontiguousarray(
            Wo[:, rows].T.reshape(HPC, HD, E).transpose(1, 0, 2))
        bq_c = np.ascontiguousarray((bq[rows] * sc).reshape(HPC, HD).T)
        bk_c = np.ascontiguousarray(bk[rows].reshape(HPC, HD).T)
        bv_c = np.ascontiguousarray(bv[rows].reshape(1, 256))
        in_maps.append({
            "xq": xqT[b], "xk": xkT[b], "xv": xvT[b],
            "wq": wq_c, "wk": wk_c, "wv": wv_c, "wo": wo_c,
            "bq": bq_c, "bk": bk_c, "bv": bv_c,
            "cwt": cwmT[b],
        })

    nc = _get_nc()
    kwargs = {}
    if PROFILE:
        import tempfile
        kwargs = dict(trace=True, tmpdir=tempfile.mkdtemp(prefix="bass_prof_"))
    res = run_bass_kernel_spmd(nc, in_maps, core_ids=list(range(NCORES)), **kwargs)
    global LAST_RESULTS
    LAST_RESULTS = res

    out = np.zeros((B, S, E), np.float32)
    attn = np.zeros((B, H, S, S), np.float32)
    for c in range(NCORES):
        b = c // 4
        hb = HPC * (c % 4)
        r = res.results[c]
        out[b] += r["outp"]
        recips = r["rsum"].reshape(HPC, S)
        awt = r["awt"]
        for i in range(HPC):
            np.multiply(awt[i].T, recips[i][:, None], out=attn[b, hb + i])
    out += bo
    return out, attn


# revision 15
# speedup vs baseline: 2.5363x; 2.5363x over previous
"""Trainium2 Bass kernel for nn_NormalizedCausalAttention.

Full (unsharded) inputs in, full outputs out. Internally shards across 8
NeuronCores: data-parallel over batch (B=2), tensor-parallel over heads
(16 heads -> 4 per core).

Per-core device program (core c -> batch b=c//4, heads hb=4*(c%4)..+4):
  Phase 1: Q/K/V projections in bf16 (fp32 PSUM accumulate).  qT/kT stored
           [64, h, S] (head dim on partitions 0-63), v stored [S, d]
           augmented with a ones column per head (so attn@V also produces
           softmax row sums).
  Phase 2: per q-tile of 512 and k-tile of 128 (causal tiles only), two
           heads (a "pair") per sweep: scoresT[k,q] = kT.T@qT (bf16 -> fp32
           PSUM) -> +causal bias (DVE, bf16 out) -> exp (ACT) -> attn@V
           accumulate in PSUM (PE); row sums ride as output row 64.
           Unnormalized bf16 exp tiles go to DRAM as fp32 (SWDGE cast) in
           [k,q] orientation; reciprocal row sums are exported.  The
           out-projection tiles for the finished q range run right after
           each q sweep (keeps PE warm, overlaps phase-3 DMA).
Host: sums the 4 per-batch partial outputs (+bo), and transposes/normalizes
the attention-weight planes (attn = (expT * recip[q]).T).

Masking: host folds attn_mask into the causal bias (NEG where masked), so
exp underflows to exactly 0.0 like the reference softmax.

Hardware notes:
 - fp32 matmuls stream at 1/4 rate on this PE; bf16 operands restore full
   rate.  PSUM accumulation stays fp32.
 - all K=64 matmuls keep both operands on partitions 0-63: alternating PE
   row groups (tile_position (0,0)/(64,0)) crashes the device on this stack.
 - walrus here accepts only ONE semaphore wait per instruction; the
   _split_multiwaits post-pass hoists extra waits onto NoOps.
"""

import sys

for _p in ("/opt/trn_rl_repo",):
    if _p not in sys.path:
        sys.path.insert(0, _p)

import ml_dtypes
import numpy as np

import concourse.bass as bass
import concourse.mybir as mybir
import concourse.tile as tile
from concourse.bass_utils import run_bass_kernel_spmd

B, S, E, H = 2, 2048, 1024, 16
HD = E // H  # 64
HPC = 4      # heads per core
NCORES = 8
NEG = -1.0e9
TEMPERATURE = 1.0
FP = mybir.dt.float32
BF = mybir.dt.bfloat16
NPBF = ml_dtypes.bfloat16
QT = 512               # q tile width in phase 2
NQT = S // QT          # 4
AF = mybir.ActivationFunctionType


def _split_multiwaits(nc):
    """Walrus here rejects >1 sem wait per instruction: hoist extras onto
    NoOps inserted right before the instruction on the same engine."""
    for f in nc.m.functions:
        for blk in f.blocks:
            out = []
            changed = False
            for inst in blk.instructions:
                si = inst.sync_info
                if si is not None and si.on_wait is not None and len(si.on_wait) > 1:
                    waits = list(si.on_wait)
                    for i, w in enumerate(waits[:-1]):
                        out.append(mybir.InstNoOp(
                            name=f"{inst.name}-wsplit{i}",
                            engine=inst.engine,
                            ins=[], outs=[],
                            sync_info=mybir.SyncInfo(on_wait=[w], on_update=[]),
                        ))
                    si.on_wait = waits[-1:]
                    inst.sync_info = si
                    changed = True
                out.append(inst)
            if changed:
                blk.instructions = out


def _nkt(qtj):
    # number of 128-wide k tiles needed for q in [QT*qtj, QT*qtj+QT)
    return (QT * qtj + QT - 1) // 128 + 1


def _build_nc(split_waits=True):
    nc = bass.Bass()

    xq = nc.dram_tensor("xq", [E, S], BF, kind="ExternalInput")   # query[b].T
    xk = nc.dram_tensor("xk", [E, S], BF, kind="ExternalInput")   # key_t[b].T
    xv = nc.dram_tensor("xv", [E, S], BF, kind="ExternalInput")   # value[b].T
    wq = nc.dram_tensor("wq", [E, 256], BF, kind="ExternalInput")  # (Wq[h]/8).T
    wk = nc.dram_tensor("wk", [E, 256], BF, kind="ExternalInput")
    wv = nc.dram_tensor("wv", [E, 256], BF, kind="ExternalInput")
    wo = nc.dram_tensor("wo", [64, HPC, E], BF, kind="ExternalInput")
    bq = nc.dram_tensor("bq", [64, HPC], FP, kind="ExternalInput")
    bk = nc.dram_tensor("bk", [64, HPC], FP, kind="ExternalInput")
    bv = nc.dram_tensor("bv", [1, 256], BF, kind="ExternalInput")
    cwt = nc.dram_tensor("cwt", [S, S], BF, kind="ExternalInput")  # masked cw[b].T

    awt = nc.dram_tensor("awt", [HPC, S, S], FP, kind="ExternalOutput")
    outp = nc.dram_tensor("outp", [S, E], FP, kind="ExternalOutput")
    rsum = nc.dram_tensor("rsum", [1, HPC * S], FP, kind="ExternalOutput")

    with tile.TileContext(nc) as tc:
        with tc.tile_pool(name="const", bufs=1) as cpool, \
             tc.tile_pool(name="qkv", bufs=1) as qkvpool:

            ones_bf = cpool.tile([1, 128], BF)
            nc.vector.memset(ones_bf, 1.0)
            ones_fp = cpool.tile([1, 128], FP)
            nc.vector.memset(ones_fp, 1.0)
            bvr_sb = cpool.tile([1, 256], BF)
            nc.sync.dma_start(out=bvr_sb, in_=bv[:, :])
            wo_sb = cpool.tile([64, HPC, E], BF)
            nc.sync.dma_start(out=wo_sb, in_=wo[:, :, :])

            qT_sb = qkvpool.tile([64, HPC, S], BF)
            kT_sb = qkvpool.tile([64, HPC, S], BF)
            v_sb = qkvpool.tile([128, S // 128, HPC * 65], BF)
            outT_sb = qkvpool.tile([64, HPC, S], BF)

            # ones columns of the augmented v (row-sum trick)
            v4 = v_sb.rearrange("p s (h x) -> p s h x", x=65)
            nc.vector.memset(v4[:, :, :, 64:65], 1.0)

            # ---------------- Phase 1: projections ----------------
            with tc.tile_pool(name="xp", bufs=2) as xpool, \
                 tc.tile_pool(name="wp", bufs=1) as wpool, \
                 tc.tile_pool(name="pp", bufs=3, space="PSUM") as ppsum:

                bq_sb = wpool.tile([64, HPC], FP, tag="bias")
                nc.sync.dma_start(out=bq_sb, in_=bq[:, :])
                bk_sb = wpool.tile([64, HPC], FP, tag="bias")
                nc.sync.dma_start(out=bk_sb, in_=bk[:, :])

                for (xdram, wdram, b_sb, dst) in (
                    (xq, wq, bq_sb, qT_sb),
                    (xk, wk, bk_sb, kT_sb),
                ):
                    w_sb = wpool.tile([128, 8, 256], BF, tag="w")
                    nc.sync.dma_start(
                        out=w_sb, in_=wdram[:, :].rearrange("(c p) n -> p c n", p=128)
                    )
                    x_sb = xpool.tile([128, 8, S], BF, tag="x")
                    nc.sync.dma_start(
                        out=x_sb, in_=xdram[:, :].rearrange("(c p) s -> p c s", p=128)
                    )
                    for h in range(HPC):
                        for st in range(4):
                            ps = ppsum.tile([64, 512], FP, tag="pqk")
                            for c in range(8):
                                nc.tensor.matmul(
                                    ps,
                                    w_sb[:, c, 64 * h:64 * h + 64],
                                    x_sb[:, c, 512 * st:512 * st + 512],
                                    start=(c == 0),
                                    stop=(c == 7),
                                )
                            nc.scalar.activation(
                                dst[:, h, 512 * st:512 * st + 512],
                                ps,
                                AF.Identity,
                                bias=b_sb[:, h:h + 1],
                                scale=1.0,
                            )

                # v projection: v[s, d] with per-head ones columns
                wv_sb = wpool.tile([128, 8, 256], BF, tag="w")
                nc.sync.dma_start(
                    out=wv_sb, in_=wv[:, :].rearrange("(c p) n -> p c n", p=128)
                )
                xv_sb = xpool.tile([128, 8, S], BF, tag="x")
                nc.sync.dma_start(
                    out=xv_sb, in_=xv[:, :].rearrange("(c p) s -> p c s", p=128)
                )
                for st in range(S // 128):
                    ps = ppsum.tile([128, 256], FP, tag="pv")
                    for c in range(8):
                        nc.tensor.matmul(
                            ps,
                            xv_sb[:, c, 128 * st:128 * st + 128],
                            wv_sb[:, c, :],
                            start=(c == 0),
                            stop=False,
                        )
                    # + bv (outer product with ones)
                    nc.tensor.matmul(ps, ones_bf, bvr_sb, start=False, stop=True)
                    nc.scalar.copy(
                        v4[:, st, :, 0:64],
                        ps[:, :].rearrange("p (h x) -> p h x", x=64),
                    )

            # ------- Phase 2: attention (+ interleaved out-projection) -------
            with tc.tile_pool(name="cw", bufs=2) as cwpool, \
                 tc.tile_pool(name="ep", bufs=2) as epool, \
                 tc.tile_pool(name="rcp", bufs=6) as rcppool, \
                 tc.tile_pool(name="rb", bufs=2) as rbpool, \
                 tc.tile_pool(name="op", bufs=3) as opool, \
                 tc.tile_pool(name="sps", bufs=2, space="PSUM") as spspool, \
                 tc.tile_pool(name="ops", bufs=3, space="PSUM") as opspool, \
                 tc.tile_pool(name="rbp", bufs=1, space="PSUM") as rbppool:

                for qtj in range(NQT):
                    nkt = _nkt(qtj)
                    q0 = QT * qtj
                    cw_t2 = cwpool.tile([128, 16, QT], BF, tag="cw", name=f"cw{qtj}")
                    nc.sync.dma_start(
                        out=cw_t2[:, 0:nkt, :],
                        in_=cwt[0:128 * nkt, q0:q0 + QT].rearrange(
                            "(k p) q -> p k q", p=128
                        ),
                    )
                    for p2 in range(2):
                        e_t = epool.tile([128, nkt, 2, QT], BF, tag="e",
                                         name=f"e{qtj}_{p2}")
                        o_ps = [opspool.tile([65, QT], FP, tag="o",
                                             name=f"o{qtj}_{p2}_{j2}")
                                for j2 in range(2)]
                        for kt in range(nkt):
                            cw_ap = cw_t2[:, kt, :]
                            cw_b2 = bass.AP(
                                tensor=cw_ap.tensor,
                                offset=cw_ap.offset,
                                ap=[list(cw_ap.ap[0]), [0, 2], list(cw_ap.ap[1])],
                            )
                            sp = spspool.tile([128, 2, QT], FP, tag="s",
                                              name=f"sp{qtj}_{kt}_{p2}")
                            for j2 in range(2):
                                h = 2 * p2 + j2
                                nc.tensor.matmul(
                                    sp[:, j2, :],
                                    kT_sb[:, h, 128 * kt:128 * kt + 128],
                                    qT_sb[:, h, q0:q0 + QT],
                                    start=True,
                                    stop=True,
                                )
                            # + causal bias, both heads at once (bf16 out)
                            nc.vector.tensor_add(e_t[:, kt, :, :], sp, cw_b2)
                            nc.scalar.activation(
                                e_t[:, kt, :, :], e_t[:, kt, :, :], AF.Exp
                            )
                            for j2 in range(2):
                                h = 2 * p2 + j2
                                nc.tensor.matmul(
                                    o_ps[j2],
                                    v_sb[:, kt, 65 * h:65 * h + 65],
                                    e_t[:, kt, j2, :],
                                    start=(kt == 0),
                                    stop=(kt == nkt - 1),
                                )
                        for j2 in range(2):
                            h = 2 * p2 + j2
                            rc = rcppool.tile([1, QT], FP, tag="rc",
                                              name=f"rc{qtj}_{h}")
                            nc.vector.reciprocal(rc, o_ps[j2][64:65, :])
                            nc.sync.dma_start(
                                out=rsum[:, h * S + q0:h * S + q0 + QT], in_=rc
                            )
                            rb_ps = rbppool.tile([128, QT], FP, tag="rb",
                                                 name=f"rb{qtj}_{h}")
                            nc.tensor.matmul(rb_ps, ones_fp, rc, start=True,
                                             stop=True)
                            rb_sb = rbpool.tile([128, QT], FP, tag="rbs",
                                                name=f"rbs{qtj}_{h}")
                            nc.scalar.copy(rb_sb, rb_ps)
                            # normalized attention output (pre out-projection)
                            nc.vector.tensor_mul(
                                outT_sb[:, h, q0:q0 + QT],
                                o_ps[j2][0:64, :],
                                rb_sb[0:64, :],
                            )
                            # unnormalized exp tiles -> DRAM fp32 (SWDGE cast)
                            nc.gpsimd.dma_start(
                                out=awt[h, 0:128 * nkt, q0:q0 + QT].rearrange(
                                    "(k p) q -> p k q", p=128
                                ),
                                in_=e_t[:, :, j2, :],
                            )

                    # out-projection for the q rows finished in this qtj
                    for st in range(4 * qtj, 4 * qtj + 4):
                        for et in range(2):
                            po = spspool.tile([128, 2, QT], FP, tag="s",
                                              name=f"po{st}_{et}")
                            ps = po[:, 0, :]
                            for h in range(HPC):
                                nc.tensor.matmul(
                                    ps,
                                    outT_sb[:, h, 128 * st:128 * st + 128],
                                    wo_sb[:, h, 512 * et:512 * et + 512],
                                    start=(h == 0),
                                    stop=(h == HPC - 1),
                                )
                            ot = opool.tile([128, 512], FP, tag="ot",
                                            name=f"ot{st}_{et}")
                            nc.scalar.copy(ot, ps)
                            nc.sync.dma_start(
                                out=outp[128 * st:128 * st + 128,
                                         512 * et:512 * et + 512],
                                in_=ot,
                            )
    if split_waits:
        _split_multiwaits(nc)
    return nc


_NC_CACHE = None
PROFILE = False           # set by test harness; harness-safe default off
LAST_RESULTS = None       # BassKernelResults of the last run (for profiling)


def _get_nc():
    global _NC_CACHE
    if _NC_CACHE is None:
        _NC_CACHE = _build_nc()
    return _NC_CACHE


def kernel(query, key_t, value, causal_weights, attn_mask,
           Wq, bq, Wk, bk, Wv, bv, Wo, bo, Wc, bc, **_unused):
    query = np.asarray(query, np.float32)
    key_t = np.asarray(key_t, np.float32)
    value = np.asarray(value, np.float32)
    causal_weights = np.asarray(causal_weights, np.float32)
    attn_mask = np.asarray(attn_mask)
    Wq = np.asarray(Wq, np.float32)
    Wk = np.asarray(Wk, np.float32)
    Wv = np.asarray(Wv, np.float32)
    Wo = np.asarray(Wo, np.float32)
    bq = np.asarray(bq, np.float32)
    bk = np.asarray(bk, np.float32)
    bv = np.asarray(bv, np.float32)
    bo = np.asarray(bo, np.float32)

    sc = 1.0 / np.sqrt(np.float32(HD))

    # per-batch shared tensors (bf16 for the matmul path)
    xqT, xkT, xvT, cwmT = {}, {}, {}, {}
    for b in range(B):
        xqT[b] = np.ascontiguousarray(query[b].T).astype(NPBF)
        xkT[b] = np.ascontiguousarray(key_t[b].T).astype(NPBF)
        xvT[b] = np.ascontiguousarray(value[b].T).astype(NPBF)
        cwm = np.where(attn_mask[b] != 0,
                       causal_weights[b] / np.float32(TEMPERATURE),
                       np.float32(NEG)).astype(np.float32)
        cwmT[b] = np.ascontiguousarray(cwm.T).astype(NPBF)

    in_maps = []
    for c in range(NCORES):
        b = c // 4
        hb = HPC * (c % 4)
        rows = slice(hb * HD, (hb + HPC) * HD)  # 256 projection dims
        wq_c = np.ascontiguousarray((Wq[rows] * sc).T).astype(NPBF)
        wk_c = np.ascontiguousarray(Wk[rows].T).astype(NPBF)
        wv_c = np.ascontiguousarray(Wv[rows].T).astype(NPBF)
        # wo[d, h, e] = Wo[e, hb*64 + h*64 + d]
        wo_c = np.ascontiguousarray(
            Wo[:, rows].T.reshape(HPC, HD, E).transpose(1, 0, 2)).astype(NPBF)
        bq_c = np.ascontiguousarray((bq[rows] * sc).reshape(HPC, HD).T)
        bk_c = np.ascontiguousarray(bk[rows].reshape(HPC, HD).T)
        bv_c = np.ascontiguousarray(bv[rows].reshape(1, 256)).astype(NPBF)
        in_maps.append({
            "xq": xqT[b], "xk": xkT[b], "xv": xvT[b],
            "wq": wq_c, "wk": wk_c, "wv": wv_c, "wo": wo_c,
            "bq": bq_c, "bk": bk_c, "bv": bv_c,
            "cwt": cwmT[b],
        })

    nc = _get_nc()
    kwargs = {}
    if PROFILE:
        import tempfile
        kwargs = dict(trace=True, tmpdir=tempfile.mkdtemp(prefix="bass_prof_"))
    res = run_bass_kernel_spmd(nc, in_maps, core_ids=list(range(NCORES)), **kwargs)
    global LAST_RESULTS
    LAST_RESULTS = res

    out = np.zeros((B, S, E), np.float32)
    attn = np.zeros((B, H, S, S), np.float32)
    for c in range(NCORES):
        b = c // 4
        hb = HPC * (c % 4)
        r = res.results[c]
        out[b] += r["outp"]
        recips = r["rsum"].reshape(HPC, S)
        awt = r["awt"]
        for i in range(HPC):
            np.multiply(awt[i].T, recips[i][:, None], out=attn[b, hb + i])
    out += bo
    return out, attn
